# revision 12
# baseline (speedup 1.0000x reference)
"""Exphydro (snow + 2-bucket soil + gamma-UH routing) Trainium2 Bass kernel.

Contract: kernel(x_phy [1095,15000,3] f32, params [15000,16] f32) -> [1095,15000] f32.
Shards the grid dim across 8 NeuronCores (1875 -> padded 1920 per core).

v2 architecture (vs v1's 24 serial ops/step):
  - snow recurrence rewritten in (pre-relu Sg, U=Om+Sg) coordinates: 6 Pool
    stt ops/step; avail = U + P - U' recovered in batch per block.
  - vadose recurrence grouped as V' = clip(V(C - c2p V) + Av - min(RP,V)):
    7 DVE ops/step, running one block behind the snow scan.
  - all series terms (ho, h1, qd2=h2+cp*phi, phi via tensor_tensor_scan)
    computed in batch [128, 128] ops, mostly on ACT with per-chunk [128,1]
    scale/bias vectors.
  - gamma-UH conv: 2 series x 15 taps as diag matmuls in fp32r (1 cyc/row at
    F>=256) on the PE, plus a 71-wide tail block on DVE.
"""
import numpy as np

T = 1095
TB = 128
NB = 9
TPAD = TB * NB           # 1152
G = 15000
NCORES = 8
GC = 1920                # padded grid per core
NCH = 15                 # chunks of 128 per core
L = 15                   # UH length
NZ = 1e-5

_CACHE = {}

# conv F-blocks: two PE groups of 512, one DVE tail of 71
FBS = [(0, 512), (512, 512), (1024, 71)]
# iteration -> list of (fb_index, chunk) PE conv units emitted there
CONV_SCHED = {
    4: [(0, c) for c in range(0, 4)],
    5: [(0, c) for c in range(4, 8)],
    6: [(0, c) for c in range(8, 12)],
    7: [(0, c) for c in range(12, 15)],
    8: [(1, c) for c in range(0, 5)],
    9: [(1, c) for c in range(5, 10)],
}
CONV_TAIL_PE = [(1, c) for c in range(10, 15)]


def _build_program():
    import concourse.bass as bass
    import concourse.mybir as mybir
    from concourse.tile import TileContext

    dt = mybir.dt
    f32 = dt.float32
    f32r = dt.float32r
    Alu = mybir.AluOpType
    Act = mybir.ActivationFunctionType

    nc = bass.Bass()

    x_t = nc.dram_tensor("x", [3, NCH, 128, TPAD], f32, kind="ExternalInput")
    pr_t = nc.dram_tensor("pr", [16, NCH, 128], f32, kind="ExternalInput")
    out_t = nc.dram_tensor("out", [NCH, 128, T], f32, kind="ExternalOutput")

    ident_np = np.eye(128, dtype=np.float32)
    tl = np.arange(L, dtype=np.float32) + 0.5
    tl_np = np.tile(tl, (128, 1))
    lntl_np = np.tile(np.log(tl), (128, 1))
    ident_t = nc.inline_tensor(ident_np, "ident")
    tl_t = nc.inline_tensor(tl_np, "tlc")
    lntl_t = nc.inline_tensor(lntl_np, "lntlc")

    SERW = 16 + TPAD + 16
    ser_t = nc.dram_tensor("ser", [2, NCH, 128, SERW], f32, kind="Internal")

    with TileContext(nc) as tc:
        with (
            tc.tile_pool(name="pers", bufs=1) as pers,
            tc.tile_pool(name="blk", bufs=1) as blk,
            tc.tile_pool(name="pv", bufs=3) as pvp,
            tc.tile_pool(name="conv", bufs=2) as convp,
            tc.tile_pool(name="dgp", bufs=2) as dgp,
            tc.tile_pool(name="psum", bufs=2, space="PSUM") as psump,
        ):
            praw = pers.tile([128, 16 * NCH], f32, tag="praw", name="praw")
            NPAR = 24
            pd = pers.tile([128, NPAR * NCH], f32, tag="pd", name="pd")
            (P_W1P, P_C2P, P_EPSV, P_TBF, P_FE, P_LNKF, P_DDF, P_NDT,
             P_NETV, P_IVM, P_NCR, P_OMCV, P_CR, P_CV, P_CP, P_DPHI,
             P_VM, P_A1M, P_IB1, P_A2M, P_IB2, P_PC2P, P_PSI0,
             P_SQPC2P) = range(NPAR)

            def pcol(j, c=None):
                if c is None:
                    return pd[:, j * NCH:(j + 1) * NCH]
                return pd[:, j * NCH + c:j * NCH + c + 1]

            ident = pers.tile([128, 128], f32, tag="ident", name="identt")
            tlt = pers.tile([128, L], f32, tag="tlt", name="tlt")
            lntlt = pers.tile([128, L], f32, tag="lntlt", name="lntlt")
            uhw = pers.tile([128, 2 * NCH * L], f32, tag="uhw", name="uhw")
            zeros16 = pers.tile([128, 16], f32, tag="z16", name="z16")
            cbias = pers.tile([128, 2], f32, tag="cbias", name="cbias")

            # scan states / scratch
            y2st = pers.tile([128, NCH], f32, tag="y2st", name="y2st")
            scp = pers.tile([128, 4 * NCH], f32, tag="scp", name="scp")  # y1|q|T4|UP
            scv = pers.tile([128, 5 * NCH], f32, tag="scv", name="scv")  # z1|z2|z3|na|z4
            uss = [pers.tile([128, NCH * (TB + 1)], f32, tag=f"uss{i}", name=f"uss{i}")
                   for i in range(2)]
            vss = pers.tile([128, NCH * (TB + 1)], f32, tag="vss", name="vss")
            phs = pers.tile([128, NCH * (TB + 1)], f32, tag="phs", name="phs")

            raw = [blk.tile([128, 3 * NCH * TB], f32, tag=f"raw{i}", name=f"raw{i}")
                   for i in range(2)]
            sst = [blk.tile([128, 2 * NCH * TB], f32, tag=f"sst{i}", name=f"sst{i}")
                   for i in range(2)]
            avst = [blk.tile([128, 2 * NCH * TB], f32, tag=f"avst{i}", name=f"avst{i}")
                    for i in range(2)]
            nrst = [blk.tile([128, NCH * TB], f32, tag=f"nrst{i}", name=f"nrst{i}")
                    for i in range(2)]
            vps = [blk.tile([128, NCH * TB], f32, tag=f"vps{i}", name=f"vps{i}")
                   for i in range(2)]
            paa = [blk.tile([128, TB], f32, tag=f"paa{i}", name=f"paa{i}") for i in range(2)]
            pab = [blk.tile([128, TB], f32, tag=f"pab{i}", name=f"pab{i}") for i in range(2)]
            pac = [blk.tile([128, TB], f32, tag=f"pac{i}", name=f"pac{i}") for i in range(2)]

            nc.sync.dma_start(ident[:], ident_t[:, :])
            nc.sync.dma_start(tlt[:], tl_t[:, :])
            nc.sync.dma_start(lntlt[:], lntl_t[:, :])
            nc.sync.dma_start(praw[:], pr_t.rearrange("j c p -> p (j c)"))
            nc.vector.memset(zeros16[:], 0.0)
            nc.vector.memset(cbias[:, 0:1], NZ)
            nc.vector.memset(cbias[:, 1:2], -1.0)

            def rawp(j):
                return praw[:, j * NCH:(j + 1) * NCH]

            ts = nc.vector.tensor_scalar
            tt = nc.vector.tensor_tensor
            stt = nc.vector.scalar_tensor_tensor
            pstt = nc.gpsimd.scalar_tensor_tensor
            pts = nc.gpsimd.tensor_scalar
            act = nc.scalar.activation

            # ---- derive params ----
            def ds(dst, j, lo, hi):
                ts(dst, rawp(j), float(hi - lo), float(lo), Alu.mult, Alu.add)

            ds(pcol(P_DDF), 0, 0.0, 40.0)
            ds(pcol(P_NDT), 1, -2.0, 3.0)                       # Tbm for now
            tt(pcol(P_NDT), pcol(P_DDF), pcol(P_NDT), Alu.mult)
            ts(pcol(P_NDT), pcol(P_NDT), -1.0, None, Alu.mult)  # -ddf*Tbm
            ds(pcol(P_W1P), 2, 1.0, 1.5)                        # 1 + wrf
            ds(pcol(P_TBF), 3, -5.0, 2.0)
            ds(pcol(P_LNKF), 4, 0.0, 5.0)                       # Kf for now
            act(pcol(P_LNKF), pcol(P_LNKF), Act.Ln)
            ts(pcol(P_LNKF), pcol(P_LNKF), -80.0, None, Alu.max)
            ds(pcol(P_FE), 5, 0.0, 1.0)
            ds(pcol(P_NETV), 6, 0.0, 1.0)                       # ETe for now
            ds(pcol(P_NCR), 7, 0.0, 1.0)                        # cr for now
            ds(pcol(P_C2P), 8, 1e-5, 0.02)
            ds(pcol(P_CV), 9, 0.0, 0.1)
            ds(pcol(P_CP), 10, 1e-5, 0.01)
            ds(pcol(P_VM), 11, 1e-3, 500.0)
            nc.vector.reciprocal(pcol(P_IVM), pcol(P_VM))
            tt(pcol(P_NETV), pcol(P_NETV), pcol(P_IVM), Alu.mult)
            ts(pcol(P_NETV), pcol(P_NETV), -1.0, None, Alu.mult)  # -ETe*ivm
            ts(pcol(P_EPSV), pcol(P_IVM), NZ, None, Alu.mult)
            ts(pcol(P_CR), pcol(P_NCR), 1.0, None, Alu.mult)      # cr
            ts(pcol(P_NCR), pcol(P_NCR), -1.0, None, Alu.mult)    # -cr
            ts(pcol(P_OMCV), pcol(P_CV), -1.0, 1.0, Alu.mult, Alu.add)  # 1-cv
            ts(pcol(P_DPHI), pcol(P_CP), -1.0, 1.0, Alu.mult, Alu.add)  # 1-cp
            tt(pcol(P_PC2P), pcol(P_CP), pcol(P_C2P), Alu.mult)   # cp*c2p
            tt(pcol(P_PSI0), pcol(P_CP), pcol(P_EPSV), Alu.mult)  # psi init
            nc.scalar.sqrt(pcol(P_SQPC2P), pcol(P_PC2P))          # sqrt(cp*c2p)
            ds(pcol(P_A1M), 12, 0.3, 20.0)
            ts(pcol(P_A1M), pcol(P_A1M), -1.0, None, Alu.add)
            ds(pcol(P_IB1), 13, 0.01, 5.0)
            nc.vector.reciprocal(pcol(P_IB1), pcol(P_IB1))
            ds(pcol(P_A2M), 14, 0.5, 13.0)
            ts(pcol(P_A2M), pcol(P_A2M), -1.0, None, Alu.add)
            ds(pcol(P_IB2), 15, 0.15, 1.5)
            nc.vector.reciprocal(pcol(P_IB2), pcol(P_IB2))

            # ---- state init ----
            nc.vector.memset(y2st[:], NZ)                        # Sg0 = NZ
            us4 = [u[:].rearrange("p (c t) -> p t c", c=NCH, t=TB + 1) for u in uss]
            vs4 = vss[:].rearrange("p (c t) -> p t c", c=NCH, t=TB + 1)
            ph4 = phs[:].rearrange("p (c t) -> p t c", c=NCH, t=TB + 1)
            nc.gpsimd.memset(us4[0][:, 0, :], 2 * NZ)            # U0 = Om0+Sg0
            ts(vs4[:, 0, :], pcol(P_EPSV), 0.0, None, Alu.add)   # V0 = eps/vm
            ts(ph4[:, 0, :], pcol(P_PSI0), 0.0, None, Alu.add)   # psi0 = cp*eps/vm

            # ---- UH weights + series zero prefixes (emitted after block 0) ----
            lgt = blk.tile([128, L], f32, tag="lgt", name="lgt")
            et = blk.tile([128, L], f32, tag="et", name="et")
            ssum = blk.tile([128, 1], f32, tag="ssum", name="ssum")

            def emit_uh_and_prefixes():
                for ui, amj in enumerate([P_A1M, P_A2M]):
                    for c in range(NCH):
                        am = pcol(amj, c)
                        ib = pcol(P_IB1, c) if ui == 0 else pcol(P_IB2, c)
                        ts(lgt[:], lntlt[:], am, None, Alu.mult)
                        stt(lgt[:], tlt[:], ib, lgt[:], Alu.mult, Alu.subtract)
                        act(et[:], lgt[:], Act.Exp, scale=-1.0)
                        nc.vector.tensor_reduce(ssum[:], et[:], mybir.AxisListType.X, Alu.add)
                        nc.vector.reciprocal(ssum[:], ssum[:])
                        ts(et[:], et[:], ssum[:], None, Alu.mult)
                        wdst = uhw[:, (ui * NCH + c) * L:(ui * NCH + c) * L + L]
                        ts(wdst, et[:], pcol(P_VM, c), None, Alu.mult)
                for c in range(NCH):
                    for si in range(2):
                        nc.sync.dma_start(ser_t[si, c, :, 0:16], zeros16[:])

            # ---- conv unit (PE, fp32r diag matmuls) ----
            def emit_conv_pe(fbi, c):
                f0, F = FBS[fbi]
                rhs = convp.tile([128, 2 * (F + 14)], f32r, tag="rhs", name="rhs")
                nc.sync.dma_start(
                    rhs[:].rearrange("p (s t) -> p s t", s=2, t=F + 14),
                    ser_t[:, c, :, 2 + f0:2 + f0 + F + 14]
                    .rearrange("s p t -> p s t").bitcast(f32r))
                ps = psump.tile([128, F], f32, tag="ps", name="ps")
                for s in range(2):
                    for l in range(L):
                        dg = dgp.tile([128, 128], f32r, tag="dg", name="dg")
                        wcol = uhw[:, (s * NCH + c) * L + l:(s * NCH + c) * L + l + 1]
                        act(dg[:], ident[:], Act.Copy, scale=wcol)
                        nc.tensor.matmul(
                            ps[:, 0:F], dg[:],
                            rhs[:, s * (F + 14) + 14 - l:s * (F + 14) + 14 - l + F],
                            start=(s == 0 and l == 0), stop=(s == 1 and l == L - 1))
                ot = convp.tile([128, F], f32, tag="ot", name="ot")
                nc.scalar.copy(ot[:], ps[:, 0:F])
                nc.sync.dma_start(out_t[c, :, f0:f0 + F], ot[:])

            # ---- conv tail unit (DVE/Pool stt accumulate) ----
            def emit_conv_dve(fbi, c, eng_stt, eng_ts):
                f0, F = FBS[fbi]
                rhs = convp.tile([128, 2 * (F + 14)], f32, tag="rhs2", name="rhs2")
                nc.sync.dma_start(
                    rhs[:].rearrange("p (s t) -> p s t", s=2, t=F + 14),
                    ser_t[:, c, :, 2 + f0:2 + f0 + F + 14].rearrange("s p t -> p s t"))
                acc = convp.tile([128, F], f32, tag="acc", name="acc")
                first = True
                for s in range(2):
                    for l in range(L):
                        wcol = uhw[:, (s * NCH + c) * L + l:(s * NCH + c) * L + l + 1]
                        rsl = rhs[:, s * (F + 14) + 14 - l:s * (F + 14) + 14 - l + F]
                        if first:
                            eng_ts(acc[:], rsl, wcol, None, Alu.mult)
                            first = False
                        else:
                            eng_stt(acc[:], rsl, wcol, acc[:], Alu.mult, Alu.add)
                nc.sync.dma_start(out_t[c, :, f0:f0 + F], acc[:])

            # ---- phase A: derive streams for block b ----
            def emit_phase_a(b):
                pa = b % 2
                rw, ss, nr = raw[pa], sst[pa], nrst[pa]
                for c in range(NCH):
                    Pc = rw[:, (c * 3 + 0) * TB:(c * 3 + 0) * TB + TB]
                    Tc = rw[:, (c * 3 + 1) * TB:(c * 3 + 1) * TB + TB]
                    Ec = rw[:, (c * 3 + 2) * TB:(c * 3 + 2) * TB + TB]
                    a_, b_, c_ = paa[c % 2], pab[c % 2], pac[c % 2]
                    PFd = ss[:, (0 * NCH + c) * TB:(0 * NCH + c) * TB + TB]
                    Ad = ss[:, (1 * NCH + c) * TB:(1 * NCH + c) * TB + TB]
                    NRd = nr[:, c * TB:c * TB + TB]
                    act(a_[:], Tc, Act.Relu, scale=-1.0, bias=pcol(P_TBF, c))
                    act(b_[:], a_[:], Act.Ln, bias=cbias[:, 0:1])
                    act(PFd, b_[:], Act.Exp, scale=pcol(P_FE, c), bias=pcol(P_LNKF, c))
                    act(a_[:], Tc, Act.Relu, scale=pcol(P_DDF, c), bias=pcol(P_NDT, c))
                    act(NRd, Ec, Act.Copy, scale=pcol(P_NETV, c))
                    stt(c_[:], Tc, 0.0, Pc, Alu.is_lt, Alu.mult)      # snow
                    tt(Ad, c_[:], a_[:], Alu.subtract)                # A = S - MP

            # ---- post-snow: avail -> Av, C streams for block b ----
            def emit_post_snow(b):
                pa = b % 2
                vs = avst[pa]
                rw = raw[pa]
                for c in range(NCH):
                    c_, b_ = pac[c % 2], pab[c % 2]
                    base = c * (TB + 1)
                    AVd = vs[:, (0 * NCH + c) * TB:(0 * NCH + c) * TB + TB]
                    Cd = vs[:, (1 * NCH + c) * TB:(1 * NCH + c) * TB + TB]
                    tt(b_[:], uss[pa][:, base:base + TB],
                       uss[pa][:, base + 1:base + 1 + TB], Alu.subtract)
                    tt(c_[:], b_[:], rw[:, (c * 3 + 0) * TB:(c * 3 + 0) * TB + TB],
                       Alu.add)
                    act(AVd, c_[:], Act.Copy, scale=pcol(P_IVM, c))
                    act(Cd, AVd, Act.Identity, scale=pcol(P_NCR, c), bias=pcol(P_OMCV, c))

            # ---- post-V: series ho/qd2 for block bb, DMA to ser ----
            def emit_post_v(bb):
                t0 = bb * TB
                vsb = avst[bb % 2]
                for c in range(NCH):
                    base = c * (TB + 1)
                    VSh = vss[:, base:base + TB]
                    AVc = vsb[:, (0 * NCH + c) * TB:(0 * NCH + c) * TB + TB]
                    t_ovf = pvp.tile([128, TB], f32, tag="tovf", name="tovf")
                    t_h1 = pvp.tile([128, TB], f32, tag="th1", name="th1")
                    t_m1 = pvp.tile([128, TB], f32, tag="tm1", name="tm1")
                    hq = pvp.tile([128, 2 * TB], f32, tag="hq", name="hq")
                    ts(t_ovf[:], vps[bb % 2][:, c * TB:c * TB + TB], -1.0, 0.0,
                       Alu.add, Alu.max)
                    act(t_h1[:], VSh, Act.Square, scale=pcol(P_SQPC2P, c))
                    if bb > 0:
                        ts(phs[:, base:base + 1], phs[:, base + TB:base + TB + 1],
                           0.0, None, Alu.add)
                    nc.vector.tensor_tensor_scan(
                        phs[:, base + 1:base + 1 + TB],
                        pcol(P_DPHI, c).broadcast_to((128, TB)), t_h1[:],
                        phs[:, base:base + 1], Alu.mult, Alu.add)
                    tt(t_m1[:], VSh, AVc, Alu.mult)
                    stt(hq[:, 0:TB], t_m1[:], pcol(P_CR, c), t_ovf[:], Alu.mult, Alu.add)
                    stt(hq[:, TB:2 * TB], VSh, pcol(P_CV, c),
                        phs[:, base:base + TB], Alu.mult, Alu.add)
                    nc.sync.dma_start(
                        ser_t[:, c, :, 16 + t0:16 + t0 + TB].rearrange("s p t -> p s t"),
                        hq[:].rearrange("p (s t) -> p s t", s=2, t=TB))

            # ---- raw input DMA for block b ----
            def emit_raw_dma(b):
                pa = b % 2
                t0 = b * TB
                for c in range(NCH):
                    nc.sync.dma_start(
                        raw[pa][:, c * 3 * TB:(c + 1) * 3 * TB].rearrange(
                            "p (ch t) -> p ch t", ch=3, t=TB),
                        x_t[:, c, :, t0:t0 + TB].rearrange("ch p t -> p ch t"))

            vps4 = [v[:].rearrange("p (c t) -> p t c", c=NCH, t=TB) for v in vps]

            def S(i):
                return scp[:, i * NCH:(i + 1) * NCH]

            def Z(i):
                return scv[:, i * NCH:(i + 1) * NCH]

            emit_raw_dma(0)
            with tc.high_priority():
                emit_phase_a(0)

            for b in range(NB + 1):
                if b + 1 < NB:
                    emit_raw_dma(b + 1)
                    with tc.high_priority():
                        emit_phase_a(b + 1)
                if b >= 1 and b < NB:
                    pts(us4[b % 2][:, 0, :], us4[(b - 1) % 2][:, TB, :],
                        0.0, None, Alu.add)
                if b >= 2:
                    ts(vs4[:, 0, :], vs4[:, TB, :], 0.0, None, Alu.add)

                sm = (sst[b % 2][:].rearrange("p (s c t) -> p t (s c)", s=2, c=NCH, t=TB)
                      if b < NB else None)
                pv = (raw[b % 2][:].rearrange("p (c s t) -> p t s c", c=NCH, s=3, t=TB)
                      if b < NB else None)
                vm4 = (avst[(b - 1) % 2][:].rearrange("p (s c t) -> p t (s c)", s=2, c=NCH, t=TB)
                       if b >= 1 else None)
                nr4 = (nrst[(b - 1) % 2][:].rearrange("p (c t) -> p t c", c=NCH, t=TB)
                       if b >= 1 else None)

                for t in range(TB):
                    if b < NB:
                        PFt = sm[:, t, 0:NCH]
                        At = sm[:, t, NCH:2 * NCH]
                        Pt = pv[:, t, 0, :]
                        Ut = us4[b % 2][:, t, :]
                        pstt(S(0), y2st[:], 0.0, PFt, Alu.max, Alu.add)       # y1
                        pstt(S(1), S(0), 0.0, Ut, Alu.max, Alu.min)           # q
                        pstt(y2st[:], S(1), 1.0, At, Alu.mult, Alu.add)       # y2'
                        pstt(S(2), y2st[:], 0.0, pcol(P_W1P), Alu.max, Alu.mult)  # T4
                        pstt(S(3), Ut, 1.0, Pt, Alu.mult, Alu.add)            # UP
                        pstt(us4[b % 2][:, t + 1, :], S(3), 0.0, S(2),
                             Alu.max, Alu.min)                                # U'
                    if b >= 1:
                        AVt = vm4[:, t, 0:NCH]
                        Ct = vm4[:, t, NCH:2 * NCH]
                        NRt = nr4[:, t, :]
                        Vt = vs4[:, t, :]
                        tt(Z(0), pcol(P_C2P), Vt, Alu.mult)                   # z1
                        tt(Z(1), Ct, Z(0), Alu.subtract)                      # z2
                        tt(Z(2), Vt, Z(1), Alu.mult)                          # z3
                        stt(Z(3), Vt, -1.0, NRt, Alu.mult, Alu.max)           # na
                        tt(Z(4), Z(2), AVt, Alu.add)                          # z4
                        tt(vps4[(b - 1) % 2][:, t, :], Z(4), Z(3), Alu.add)   # Vp
                        stt(vs4[:, t + 1, :], vps4[(b - 1) % 2][:, t, :], 1.0,
                            pcol(P_EPSV), Alu.min, Alu.max)                   # V'

                if b < NB:
                    with tc.high_priority():
                        emit_post_snow(b)
                if b >= 1:
                    emit_post_v(b - 1)
                if b == 0:
                    emit_uh_and_prefixes()
                for fbi, c in CONV_SCHED.get(b, []):
                    emit_conv_pe(fbi, c)

            for c in range(NCH):
                if c < 8:
                    emit_conv_dve(2, c, stt, ts)
                else:
                    emit_conv_dve(2, c, pstt, pts)
            for fbi, c in CONV_TAIL_PE:
                emit_conv_pe(fbi, c)

    _strip_same_engine_waits(nc)
    _split_multi_waits(nc)
    return nc


def _strip_same_engine_waits(nc):
    """Drop semaphore waits that only order an engine against itself.

    Engines execute their instruction queue in order, so a wait on a sem
    whose every updater is a non-DMA instruction on the same engine is
    redundant ordering-wise; TimelineSim charges ~95ns per such wait
    (producer side-effect drain + sem propagation).  Cross-engine waits and
    DMA-completion waits (sems updated by DMA-class instructions, which fire
    at transfer completion, not instruction retirement) are kept.
    """
    upd_engines = {}
    dma_like = ("DMA", "TriggerDma")

    def sem_key(x):
        return (x.sync_type, x.id)

    for f in nc.m.functions:
        for bb in f.blocks:
            for ins in bb.instructions:
                si = ins.sync_info
                if si is None or not si.on_update:
                    continue
                is_dma = any(s in ins.opcode for s in dma_like)
                for u in si.on_update:
                    upd_engines.setdefault(sem_key(u), set()).add(
                        "DMA" if is_dma else ins.engine)

    stripped = 0
    for f in nc.m.functions:
        for bb in f.blocks:
            for ins in bb.instructions:
                si = ins.sync_info
                if si is None or not si.on_wait:
                    continue
                keep = []
                for w in si.on_wait:
                    engs = upd_engines.get(sem_key(w), {"?"})
                    if engs == {ins.engine}:
                        stripped += 1
                    else:
                        keep.append(w)
                if len(keep) != len(si.on_wait):
                    si.on_wait = keep


def _split_multi_waits(nc):
    """This container's walrus codegen accepts at most ONE sync wait per
    instruction; Tile emits several.  Hoist the excess onto same-engine
    NoOp carriers inserted immediately before."""
    from bass_rust import InstNoOp, SyncInfo

    cnt = 0
    for f in nc.m.functions:
        for bb in f.blocks:
            out = []
            changed = False
            for ins in bb.instructions:
                si = ins.sync_info
                w = list(si.on_wait) if si is not None and si.on_wait else []
                if len(w) > 1:
                    for extra in w[:-1]:
                        cnt += 1
                        nop = InstNoOp(name=f"WQ-{cnt}", engine=ins.engine)
                        nop.sync_info = SyncInfo(on_wait=[extra], on_update=[])
                        out.append(nop)
                    si.on_wait = [w[-1]]
                    changed = True
                out.append(ins)
            if changed:
                bb.instructions = out


def _get_program():
    if "nc" not in _CACHE:
        _CACHE["nc"] = _build_program()
    return _CACHE["nc"]


def kernel(x_phy: np.ndarray, params: np.ndarray) -> np.ndarray:
    from concourse.bass_utils import run_bass_kernel_spmd

    nc = _get_program()

    x_phy = np.ascontiguousarray(x_phy, dtype=np.float32)
    params = np.ascontiguousarray(params, dtype=np.float32)

    GPAD = NCORES * GC
    xp = np.zeros((TPAD, GPAD, 3), np.float32)
    xp[:T, :G] = x_phy
    pp = np.full((GPAD, 16), 0.5, np.float32)
    pp[:G] = params

    in_maps = []
    for k in range(NCORES):
        g0 = k * GC
        xk = np.ascontiguousarray(
            xp[:, g0:g0 + GC].transpose(2, 1, 0).reshape(3, NCH, 128, TPAD))
        pk = np.ascontiguousarray(
            pp[g0:g0 + GC].reshape(NCH, 128, 16).transpose(2, 0, 1))
        in_maps.append({"x": xk, "pr": pk})

    res = run_bass_kernel_spmd(nc, in_maps, core_ids=list(range(NCORES)))

    out = np.empty((T, G), np.float32)
    for k in range(NCORES):
        o = res.results[k]["out"]            # [NCH,128,T]
        g0 = k * GC
        hi = min(g0 + GC, G)
        flat = o.transpose(2, 0, 1).reshape(T, GC)
        out[:, g0:hi] = flat[:, :hi - g0]
    return out
